# revision 13
# baseline (speedup 1.0000x reference)
"""WENO5 2D advection (Advection3D) Trainium2 kernel — bf16 pipeline.

Full inputs h, u, v: [32, 1024, 1024] f32.  Output: same shape;
out[1:-1, 2:-2, 2:-2] = -div(WENO5 fluxes), 0 on the frame.

Sharding: z-levels across 8 cores (pure data parallel, no halo in z).
Per-core SPMD program processes ZPC=4 z-levels; each z-level swept in
y-chunks of 128 rows (122 valid output rows per chunk).

Implementation notes (perf-critical):
  - Everything elementwise runs in bf16 so DVE tensor_tensor ops hit the
    2x_1p perf mode.  That mode requires every operand 4-byte aligned, so
    each SBUF array gets a storage "phase" (0 or 1 column shift) chosen so
    all hot ops read/write even byte offsets.  Where one array is needed at
    both parities (Dx, the x-direction PPs, fe), a DMA engine makes the
    +1-column realigned copy (DMA is far from saturated).
  - y-direction stencils/shifts via TensorE banded 128x128 matmuls (bf16,
    1 cyc/row); every PSUM result is copied (often fused with Square) to
    bf16 SBUF by ScalarE so the DVE chain stays in 2x mode.
  - Reciprocal = exp(-ln(x)) on ScalarE, no Newton step (gate is 2e-2;
    table accuracy ~1e-6 relative).
  - Upwind flux via f = relu(v)*qL - relu(-v)*qR (2 ACT + 3 DVE).
  - q shifted one row (for qR's q_{j+1}) comes from a row-offset DMA of
    the same DRAM buffer, not a PE pass.
Measured numerics of this exact op chain in bf16 (numpy prototype):
rel err ~6e-3 vs f64 reference (gate 2e-2).

Math restructure (validated against reference):
  D_j = q_{j+1}-q_j ; A_j = D_j - D_{j-1}
  b0_j ~ c1312*A^2 + .25*(A+2D_j)^2 ; b1 ~ c1312*A^2+.25*(D_j+D_{j-1})^2
  b2 ~ c1312*A^2 + .25*(A-2D_{j-1})^2 ; B_k = (eps+b_k)^2
  PP12_j=B1_j*B2_{j+1}; PP01_j=B0_{j-1}*B1_j; PP02_j=B0_{j-1}*B2_{j+1}
  denL*10 = PP12+6*PP02+3*PP01 ; denR*10 = PP01+6*PP02+3*PP12 (R at i+1)
  numL*12 = PP12*dl0L + 2.4*(PP02*dl1L + PP01*dl2L)
  numR*12 = PP01'*dl0R + 2.4*(PP02'*dl1R + PP12'*dl2R)   (' = at i+1)
  qL = q_i + (5/6)*numL/denL ; qR = q_{i+1} - (5/6)*numR/denR
  flux = relu(vel)*qL - relu(-vel)*qR
"""
import math

import numpy as np

import concourse.bass as bass
import concourse.mybir as mybir
import concourse.tile as tile

F32 = mybir.dt.float32
BF = mybir.dt.bfloat16
ALU = mybir.AluOpType
AF = mybir.ActivationFunctionType

NZ, NY, NX = 32, 1024, 1024
NCORES = 8
ZPC = 4                      # z-levels per core (SPMD-uniform)
PY, PX = NY + 2, NX + 2      # edge-padded
TW = 1028                    # tile width (PX rounded up to even + slack)
DX = 1000.0
DY = 1000.0
WENO_EPS = 1e-6
C1312S = math.sqrt(13.0 / 12.0)
CHUNK = 122                  # valid output rows per 128-row chunk


class LegalTileContext(tile.TileContext):
    """Tile + wait legalization: this walrus packs at most ONE semaphore wait
    per instruction; hoist extras onto standalone EventSemaphore instructions
    (what raw-bass wait_ge emits)."""

    def _commit_instruction(self, inst, lazy_reg_writes=True):
        si = inst.sync_info
        if si is not None and len(si.on_wait) > 1:
            waits = list(si.on_wait)
            for w in waits[:-1]:
                ev = mybir.InstEventSemaphore(
                    name=f"W-{self.nc.next_id()}", ins=[], outs=[]
                )
                ev.engine = inst.engine
                ev.sync_info = mybir.SyncInfo(on_wait=[w], on_update=[])
                if inst.debug is not None:
                    ev.debug = inst.debug
                super()._commit_instruction(ev, lazy_reg_writes=False)
            inst.sync_info = mybir.SyncInfo(
                on_wait=[waits[-1]], on_update=list(si.on_update)
            )
        return super()._commit_instruction(inst, lazy_reg_writes)

    def _drain_and_barrier(self, tick_clock, wait_clock):
        from concourse.vector_clock import ScopedClock

        nop0 = self.nc.sync.nop()
        wait_clock.add_sem_waits(
            nop0.ins, ScopedClock({None: tick_clock.global_clock})
        )
        si = nop0.ins.sync_info
        if si is not None and len(si.on_wait) > 1:
            waits = list(si.on_wait)
            nop0.ins.sync_info = mybir.SyncInfo(
                on_wait=[waits[0]], on_update=list(si.on_update)
            )
            for w in waits[1:]:
                nopk = self.nc.sync.nop()
                nopk.ins.sync_info = mybir.SyncInfo(on_wait=[w], on_update=[])
        self.nc.sync.drain()

        self.nc.all_engine_barrier()
        assert self.sems is not None
        popped = self.nc._tile_sem_poison_stack.pop()
        assert popped is self._sem_poison
        self.nc.clear_and_free_semaphores(list(self.sems.allocated().values()))
        self.nc.all_engine_barrier()


class Scratch:
    """Free-list scratch allocator.  Tags are reused only after an explicit
    free(), which callers place after the tile's last consumer is emitted —
    so slot-wait edges always point backward in emission order and can
    never form a scheduling cycle."""

    def __init__(self, pool, shape, dtype, prefix="s"):
        self.pool = pool
        self.shape = shape
        self.dtype = dtype
        self.prefix = prefix
        self.free_tags = []
        self.n = 0
        self.tag_of = {}

    def __call__(self):
        tag = self.free_tags.pop() if self.free_tags else f"{self.prefix}{self._new()}"
        t = self.pool.tile(self.shape, self.dtype, tag=tag)
        self.tag_of[id(t)] = tag
        return t

    def _new(self):
        self.n += 1
        return self.n - 1

    def free(self, *tiles):
        for t in tiles:
            self.free_tags.append(self.tag_of.pop(id(t)))


# Band matrices (lhsT layout: S[k, p] = coeff of q_k in out_p).
# Edge rows are garbage (partial sums), discarded by the final DMA row range.
BAND_SPECS = [
    ("shp1", {1: 1.0}),                      # 0: out_p = q_{p+1}
    ("ay", {-1: 1.0, 0: -2.0, 1: 1.0}),      # 1: A_p
    ("t0", {-1: 1.0, 0: -4.0, 1: 3.0}),      # 2: t0_p
    ("t1", {-1: 3.0, 0: -4.0, 1: 1.0}),      # 3: t1_p
    ("s", {-1: -1.0, 1: 1.0}),               # 4: s_p
    ("dl0L", {-2: 0.4, -1: -1.4, 0: 1.0}),   # 5
    ("dl1L", {-1: -0.5, 0: -0.5, 1: 1.0}),   # 6
    ("dl2L", {0: -1.0, 1: 1.25, 2: -0.25}),  # 7
    ("dl0R", {1: -1.0, 2: 1.4, 3: -0.4}),    # 8
    ("dl1R", {0: -1.0, 1: 0.5, 2: 0.5}),     # 9
    ("dl2R", {-1: 0.25, 0: -1.25, 1: 1.0}),  # 10
    ("shm1", {-1: 1.0}),                     # 11: out_p = q_{p-1}
]
NBANDS = len(BAND_SPECS)


def make_bands_host():
    """SBUF-layout band matrices: [128 k-partitions, NBANDS*128 cols], bf16."""
    w = np.zeros((128, NBANDS * 128), dtype=np.float32)
    for b, (_, taps) in enumerate(BAND_SPECS):
        for off, coef in taps.items():
            for p in range(128):
                k = p + off
                if 0 <= k < 128:
                    w[k, b * 128 + p] = coef
    return w.astype(mybir.dt.np(BF))


YW = 1024  # y-chain logical column count


def _emit_direction_x(nc, sc, scf, wk, Qe, Qo, U):
    """X-direction WENO flux + divergence part (free-dim shifts, bf16).

    Every array stores logical column j at tile column j+phase; phases are
    chosen so every DVE op's operands start at even columns (2x_1p mode).
    Returns dfex tile (phase 1, valid logical cols [3, 1023))."""
    tt = nc.vector.tensor_tensor
    stt = nc.vector.scalar_tensor_tensor
    act = nc.scalar.activation
    dma = nc.sync.dma_start

    W = PX  # 1026 logical columns

    # Dx[j] = Q[j+1] - Q[j], j in [0, 1025); phase 0.
    Dx = sc()
    tt(Dx[:, 0 : W - 1], Qo[:, 2 : W + 1], Qe[:, 0 : W - 1], ALU.subtract)
    # Dxo: phase-1 realigned copy (DMA engine; off DVE).
    Dxo = sc()
    dma(Dxo[:, 1:W], Dx[:, 0 : W - 1])

    # Ax[j] = Dx[j] - Dx[j-1], j in [1, 1025); phase 1 (cols [2, 1026)).
    Ax = sc()
    tt(Ax[:, 2:W], Dxo[:, 2:W], Dx[:, 0 : W - 2], ALU.subtract)
    # t0 = 2*Dx[j] + Ax[j]; t1 = -2*Dx[j-1] + Ax[j]; s = Dx[j] + Dx[j-1]
    t0 = sc()
    stt(t0[:, 2:W], Dxo[:, 2:W], 2.0, Ax[:, 2:W], ALU.mult, ALU.add)
    t1 = sc()
    stt(t1[:, 2:W], Dx[:, 0 : W - 2], -2.0, Ax[:, 2:W], ALU.mult, ALU.add)
    s = sc()
    tt(s[:, 2:W], Dxo[:, 2:W], Dx[:, 0 : W - 2], ALU.add)

    # squares (ACT), phase 1 outputs
    asq = sc()
    act(asq[:, 2:W], Ax[:, 2:W], AF.Square, scale=C1312S)
    sc.free(Ax)
    q0 = sc()
    act(q0[:, 2:W], t0[:, 2:W], AF.Square, scale=0.5)
    q1 = sc()
    act(q1[:, 2:W], s[:, 2:W], AF.Square, scale=0.5)
    q2 = sc()
    act(q2[:, 2:W], t1[:, 2:W], AF.Square, scale=0.5)
    sc.free(t0, t1, s)

    # c_k = (asq + eps) + q_k, phase 1, j in [1, 1025)
    c0 = sc()
    stt(c0[:, 2:W], asq[:, 2:W], WENO_EPS, q0[:, 2:W], ALU.add, ALU.add)
    c1 = sc()
    stt(c1[:, 2:W], asq[:, 2:W], WENO_EPS, q1[:, 2:W], ALU.add, ALU.add)
    c2 = sc()
    stt(c2[:, 2:W], asq[:, 2:W], WENO_EPS, q2[:, 2:W], ALU.add, ALU.add)
    sc.free(asq, q0, q1, q2)
    # B_k = c_k^2 (ACT): B0 phase 1, B1 phase 0, B2 phase 1
    B0 = sc()
    act(B0[:, 2:W], c0[:, 2:W], AF.Square)
    B1 = sc()
    act(B1[:, 1 : W - 1], c1[:, 2:W], AF.Square)
    B2 = sc()
    act(B2[:, 2:W], c2[:, 2:W], AF.Square)
    sc.free(c0, c1, c2)

    # PP's at j in [2, 1024), phase 0:
    #   PP12 = B1[j]*B2[j+1]; PP01 = B0[j-1]*B1[j]; PP02 = B0[j-1]*B2[j+1]
    PP12 = sc()
    tt(PP12[:, 2 : W - 2], B1[:, 2 : W - 2], B2[:, 4 : W], ALU.mult)
    PP01 = sc()
    tt(PP01[:, 2 : W - 2], B0[:, 2 : W - 2], B1[:, 2 : W - 2], ALU.mult)
    PP02 = sc()
    tt(PP02[:, 2 : W - 2], B0[:, 2 : W - 2], B2[:, 4 : W], ALU.mult)
    sc.free(B0, B1, B2)
    # phase-1 realigned copies for the R-side reads at j+1 (DMA engine)
    PP12o = sc()
    dma(PP12o[:, 3 : W - 1], PP12[:, 2 : W - 2])
    PP01o = sc()
    dma(PP01o[:, 3 : W - 1], PP01[:, 2 : W - 2])
    PP02o = sc()
    dma(PP02o[:, 3 : W - 1], PP02[:, 2 : W - 2])

    # denominators, phase 0, j in [2, 1024)
    d1 = sc()
    stt(d1[:, 2 : W - 2], PP02[:, 2 : W - 2], 6.0, PP12[:, 2 : W - 2], ALU.mult, ALU.add)
    denL = sc()
    stt(denL[:, 2 : W - 2], PP01[:, 2 : W - 2], 3.0, d1[:, 2 : W - 2], ALU.mult, ALU.add)
    d2 = sc()
    stt(d2[:, 2 : W - 2], PP02[:, 2 : W - 2], 6.0, PP01[:, 2 : W - 2], ALU.mult, ALU.add)
    denR = sc()
    stt(denR[:, 2 : W - 2], PP12[:, 2 : W - 2], 3.0, d2[:, 2 : W - 2], ALU.mult, ALU.add)
    sc.free(d1, d2)

    # dl terms at faces i in [2, 1023), phase 0
    lo, hi = 2, W - 3
    F = slice(lo, hi)  # phase-0 face window (cols == logical)

    dl0L = sc()
    stt(dl0L[:, F], Dx[:, lo - 2 : hi - 2], -0.4, Dxo[:, lo : hi], ALU.mult, ALU.add)
    dl1L = sc()
    stt(dl1L[:, F], Dxo[:, lo : hi], 0.5, Dx[:, lo:hi], ALU.mult, ALU.add)
    dl2L = sc()
    stt(dl2L[:, F], Dxo[:, lo + 2 : hi + 2], -0.25, Dx[:, lo:hi], ALU.mult, ALU.add)
    dl0R = sc()
    stt(dl0R[:, F], Dx[:, lo + 2 : hi + 2], -0.4, Dxo[:, lo + 2 : hi + 2], ALU.mult, ALU.add)
    dl1R = sc()
    stt(dl1R[:, F], Dxo[:, lo + 2 : hi + 2], 0.5, Dx[:, lo:hi], ALU.mult, ALU.add)
    dl2R = sc()
    stt(dl2R[:, F], Dxo[:, lo : hi], -0.25, Dx[:, lo:hi], ALU.mult, ALU.add)
    sc.free(Dx, Dxo)

    # numerators: gL from PP at i, gR from PPo at i+1 (cols i+2: even)
    g0L = sc(); tt(g0L[:, F], PP12[:, F], dl0L[:, F], ALU.mult)
    g1L = sc(); tt(g1L[:, F], PP02[:, F], dl1L[:, F], ALU.mult)
    g2L = sc(); tt(g2L[:, F], PP01[:, F], dl2L[:, F], ALU.mult)
    sc.free(dl0L, dl1L, dl2L)
    n1L = sc(); tt(n1L[:, F], g1L[:, F], g2L[:, F], ALU.add)
    numL = sc(); stt(numL[:, F], n1L[:, F], 2.4, g0L[:, F], ALU.mult, ALU.add)
    sc.free(g0L, g1L, g2L, n1L)
    Fp1 = slice(lo + 2, hi + 2)  # phase-1 tile cols of logical i+1
    g0R = sc(); tt(g0R[:, F], PP01o[:, Fp1], dl0R[:, F], ALU.mult)
    g1R = sc(); tt(g1R[:, F], PP02o[:, Fp1], dl1R[:, F], ALU.mult)
    g2R = sc(); tt(g2R[:, F], PP12o[:, Fp1], dl2R[:, F], ALU.mult)
    sc.free(dl0R, dl1R, dl2R, PP12, PP01, PP02, PP12o, PP01o, PP02o)
    n1R = sc(); tt(n1R[:, F], g1R[:, F], g2R[:, F], ALU.add)
    numR = sc(); stt(numR[:, F], n1R[:, F], 2.4, g0R[:, F], ALU.mult, ALU.add)
    sc.free(g0R, g1R, g2R, n1R)

    # reciprocals: rdL phase 0 at [2,1023); rdR phase 1 at [2,1024)
    e = slice(2, W - 2)
    lnL = sc(); act(lnL[:, F], denL[:, F], AF.Ln)
    rdL = sc(); act(rdL[:, F], lnL[:, F], AF.Exp, scale=-1.0)
    lnR = sc(); act(lnR[:, e], denR[:, e], AF.Ln)
    rdR = sc(); act(rdR[:, 3 : W - 1], lnR[:, e], AF.Exp, scale=-1.0)
    sc.free(lnL, lnR, denL, denR)

    # face values
    tL = sc(); tt(tL[:, F], numL[:, F], rdL[:, F], ALU.mult)
    rL = sc(); stt(rL[:, F], tL[:, F], 5.0 / 6.0, Qe[:, F], ALU.mult, ALU.add)
    sc.free(numL, rdL, tL)
    tR = sc(); tt(tR[:, F], numR[:, F], rdR[:, Fp1], ALU.mult)
    rR = sc(); stt(rR[:, F], tR[:, F], -5.0 / 6.0, Qo[:, Fp1], ALU.mult, ALU.add)
    sc.free(numR, rdR, tR)

    # upwind flux: fe = relu(U)*qL - relu(-U)*qR.  The flux tail runs in
    # f32: fe differences are ~10x smaller than fe itself, so bf16-rounding
    # fe would dominate the output error budget.
    pU = sc(); act(pU[:, F], U[:, F], AF.Relu)
    nU = sc(); act(nU[:, F], U[:, F], AF.Relu, scale=-1.0)
    fa = scf(); tt(fa[:, F], pU[:, F], rL[:, F], ALU.mult)
    fb = scf(); tt(fb[:, F], nU[:, F], rR[:, F], ALU.mult)
    sc.free(pU, nU, rL, rR)
    fe = scf(); tt(fe[:, F], fa[:, F], fb[:, F], ALU.subtract)
    scf.free(fa, fb)

    # U pre-scaled by 1/DX on host; reversed diff = negated contribution:
    # dfex[i] = fe[i-1] - fe[i], i in [3, 1023).
    dfex = wk.tile([128, TW], F32, tag="dfex")
    tt(dfex[:, 3 : W - 3], fe[:, 2 : W - 4], fe[:, 3 : W - 3], ALU.subtract)
    scf.free(fe)
    return dfex


def _emit_direction_y(nc, sc, scf, wk, psc, bands, Qe, Qo, Qs1, V_):
    """Y-direction WENO flux via TensorE banded matmuls (bf16); every PSUM
    result goes through ScalarE (square/copy fused) into phase-1 bf16 SBUF
    so the DVE chain runs at 2x.  Returns dfny (phase 1, logical [1,1024))."""
    tt = nc.vector.tensor_tensor
    stt = nc.vector.scalar_tensor_tensor
    act = nc.scalar.activation

    A1 = slice(2, YW + 1)   # phase-1 cols of logical [1, 1024)
    AF0 = slice(0, YW)      # psum full window

    def pe(src, base, b):
        """Banded partition-stencil matmul.  rhs reads src cols starting at
        `base`; psum col c holds logical column base-1+c of the result for
        phase-1 sources (base=2: reads exactly the valid cols [2,1025),
        second matmul 511 wide), or logical c for phase-0 (base=0)."""
        pt = psc()
        w2 = 512 if base == 0 else 511
        nc.tensor.matmul(
            pt[:, 0:512],
            bands[:, b * 128 : (b + 1) * 128],
            src[:, base : base + 512],
        )
        nc.tensor.matmul(
            pt[:, 512 : 512 + w2],
            bands[:, b * 128 : (b + 1) * 128],
            src[:, base + 512 : base + 512 + w2],
        )
        return pt

    def cp1(p, func=AF.Copy, scale=1.0, shift=0):
        """ACT: psum -> phase-1 bf16 SBUF tile.  shift=0 for psum col ==
        logical (phase-0 source, 1024 wide); shift=1 when psum col c is
        logical c+1 (phase-1 source at base=2, 1023 valid cols)."""
        t = sc()
        w = YW - shift
        act(t[:, 1 + shift : 1 + shift + w], p[:, 0:w], func, scale=scale)
        return t

    p = pe(Qe, 0, 1)
    asq = cp1(p, AF.Square, C1312S)
    psc.free(p)
    p = pe(Qe, 0, 2)
    q0 = cp1(p, AF.Square, 0.5)
    psc.free(p)
    p = pe(Qe, 0, 3)
    q2 = cp1(p, AF.Square, 0.5)
    psc.free(p)
    p = pe(Qe, 0, 4)
    q1 = cp1(p, AF.Square, 0.5)
    psc.free(p)
    dls = []
    for b in (5, 6, 7, 8, 9, 10):
        p = pe(Qe, 0, b)
        dls.append(cp1(p))
        psc.free(p)
    dl0L, dl1L, dl2L, dl0R, dl1R, dl2R = dls

    c0 = sc(); stt(c0[:, A1], asq[:, A1], WENO_EPS, q0[:, A1], ALU.add, ALU.add)
    c1 = sc(); stt(c1[:, A1], asq[:, A1], WENO_EPS, q1[:, A1], ALU.add, ALU.add)
    c2 = sc(); stt(c2[:, A1], asq[:, A1], WENO_EPS, q2[:, A1], ALU.add, ALU.add)
    sc.free(asq, q0, q1, q2)
    B0 = sc(); act(B0[:, A1], c0[:, A1], AF.Square)
    B1 = sc(); act(B1[:, A1], c1[:, A1], AF.Square)
    B2 = sc(); act(B2[:, A1], c2[:, A1], AF.Square)
    sc.free(c0, c1, c2)
    p = pe(B0, 2, 11)
    B0m1 = cp1(p, shift=1)
    psc.free(p)
    p = pe(B2, 2, 0)
    B2p1 = cp1(p, shift=1)
    psc.free(p)
    PP12 = sc(); tt(PP12[:, A1], B1[:, A1], B2p1[:, A1], ALU.mult)
    PP01 = sc(); tt(PP01[:, A1], B0m1[:, A1], B1[:, A1], ALU.mult)
    PP02 = sc(); tt(PP02[:, A1], B0m1[:, A1], B2p1[:, A1], ALU.mult)
    sc.free(B0, B1, B2, B0m1, B2p1)
    d1 = sc()
    stt(d1[:, A1], PP02[:, A1], 6.0, PP12[:, A1], ALU.mult, ALU.add)
    denL = sc()
    stt(denL[:, A1], PP01[:, A1], 3.0, d1[:, A1], ALU.mult, ALU.add)
    d2 = sc()
    stt(d2[:, A1], PP02[:, A1], 6.0, PP01[:, A1], ALU.mult, ALU.add)
    denR = sc()
    stt(denR[:, A1], PP12[:, A1], 3.0, d2[:, A1], ALU.mult, ALU.add)
    sc.free(d1, d2)

    lnL = sc(); act(lnL[:, A1], denL[:, A1], AF.Ln)
    rdL = sc(); act(rdL[:, A1], lnL[:, A1], AF.Exp, scale=-1.0)
    lnR = sc(); act(lnR[:, A1], denR[:, A1], AF.Ln)
    rdR = sc(); act(rdR[:, A1], lnR[:, A1], AF.Exp, scale=-1.0)
    sc.free(lnL, lnR, denL, denR)

    g0L = sc(); tt(g0L[:, A1], PP12[:, A1], dl0L[:, A1], ALU.mult)
    g1L = sc(); tt(g1L[:, A1], PP02[:, A1], dl1L[:, A1], ALU.mult)
    g2L = sc(); tt(g2L[:, A1], PP01[:, A1], dl2L[:, A1], ALU.mult)
    sc.free(dl0L, dl1L, dl2L)
    n1L = sc(); tt(n1L[:, A1], g1L[:, A1], g2L[:, A1], ALU.add)
    numL = sc(); stt(numL[:, A1], n1L[:, A1], 2.4, g0L[:, A1], ALU.mult, ALU.add)
    sc.free(g0L, g1L, g2L, n1L)

    # R-side: PP's and rdR shifted one partition down (band 0), via ACT
    p = pe(PP01, 2, 0)
    PP01s = cp1(p, shift=1)
    psc.free(p)
    p = pe(PP02, 2, 0)
    PP02s = cp1(p, shift=1)
    psc.free(p)
    p = pe(PP12, 2, 0)
    PP12s = cp1(p, shift=1)
    psc.free(p)
    sc.free(PP12, PP01, PP02)
    p = pe(rdR, 2, 0)
    rdRs = cp1(p, shift=1)
    psc.free(p)
    sc.free(rdR)
    g0R = sc(); tt(g0R[:, A1], PP01s[:, A1], dl0R[:, A1], ALU.mult)
    g1R = sc(); tt(g1R[:, A1], PP02s[:, A1], dl1R[:, A1], ALU.mult)
    g2R = sc(); tt(g2R[:, A1], PP12s[:, A1], dl2R[:, A1], ALU.mult)
    sc.free(dl0R, dl1R, dl2R, PP01s, PP02s, PP12s)
    n1R = sc(); tt(n1R[:, A1], g1R[:, A1], g2R[:, A1], ALU.add)
    numR = sc(); stt(numR[:, A1], n1R[:, A1], 2.4, g0R[:, A1], ALU.mult, ALU.add)
    sc.free(g0R, g1R, g2R, n1R)

    tL = sc(); tt(tL[:, A1], numL[:, A1], rdL[:, A1], ALU.mult)
    rL = sc(); stt(rL[:, A1], tL[:, A1], 5.0 / 6.0, Qo[:, A1], ALU.mult, ALU.add)
    sc.free(numL, rdL, tL)
    tR = sc(); tt(tR[:, A1], numR[:, A1], rdRs[:, A1], ALU.mult)
    rR = sc(); stt(rR[:, A1], tR[:, A1], -5.0 / 6.0, Qs1[:, A1], ALU.mult, ALU.add)
    sc.free(numR, rdRs, tR)

    # upwind flux: fn = relu(V)*qL - relu(-V)*qR  (V pre-scaled by 1/DY).
    # f32 tail for the same rounding reason as in x.
    pV = sc(); act(pV[:, A1], V_[:, 1:YW], AF.Relu)
    nV = sc(); act(nV[:, A1], V_[:, 1:YW], AF.Relu, scale=-1.0)
    fa = scf(); tt(fa[:, A1], pV[:, A1], rL[:, A1], ALU.mult)
    fb = scf(); tt(fb[:, A1], nV[:, A1], rR[:, A1], ALU.mult)
    sc.free(pV, nV, rL, rR)
    fn = scf(); tt(fn[:, A1], fa[:, A1], fb[:, A1], ALU.subtract)
    scf.free(fa, fb)

    # dfny[p] = fn[p-1] - fn[p]: partition shift via SBUF->SBUF DMA
    fnm1 = scf()
    nc.sync.dma_start(fnm1[1:128, A1], fn[0:127, A1])
    nc.sync.dma_start(fnm1[0:1, A1], fn[0:1, A1])  # row 0: dummy fill
    dfny = scf()
    tt(dfny[:, A1], fnm1[:, A1], fn[:, A1], ALU.subtract)
    scf.free(fn, fnm1)
    return dfny


def build_nc(zpc=ZPC, n_chunks=9, mode="full", repeat=1):
    nc = bass.Bass()
    h_ext = nc.declare_dram_parameter("h", [zpc, PY, PX], BF, isOutput=False)
    u_ext = nc.declare_dram_parameter("u", [zpc, PY, PX], BF, isOutput=False)
    v_ext = nc.declare_dram_parameter("v", [zpc, PY, PX], BF, isOutput=False)
    b_ext = nc.declare_dram_parameter(
        "bands", [128, NBANDS * 128], BF, isOutput=False
    )
    o_ext = nc.declare_dram_parameter("o", [zpc, NY, NX], F32, isOutput=True)

    with LegalTileContext(nc) as tc:
        with (
            tc.tile_pool(name="inp", bufs=3) as inp,
            tc.tile_pool(name="wk", bufs=2) as wk,
            tc.tile_pool(name="outp", bufs=2) as outp,
            tc.tile_pool(name="bnd", bufs=1) as bnd,
            tc.tile_pool(name="ps", bufs=3, space="PSUM") as psum,
        ):
            bands = bnd.tile([128, NBANDS * 128], BF, tag="bands")
            nc.sync.dma_start(bands[:], b_ext[:])
            sc = Scratch(wk, [128, TW], BF)
            scf = Scratch(wk, [128, TW], F32, prefix="f")
            psc = Scratch(psum, [128, YW], F32, prefix="p")
            for _rep in range(repeat):
              for z in range(zpc):
                for ci in range(n_chunks):
                    r0 = CHUNK * ci
                    if r0 + 128 > PY:
                        r0 = PY - 128
                    Qe = inp.tile([128, TW], BF, tag="Qe")
                    nc.sync.dma_start(Qe[:, 0:PX], h_ext[z, r0 : r0 + 128, :])
                    Qo = inp.tile([128, TW], BF, tag="Qo")
                    nc.sync.dma_start(Qo[:, 1 : PX + 1], h_ext[z, r0 : r0 + 128, :])
                    # q shifted one ROW down (for qR along y), phase 1
                    Qs1 = inp.tile([128, TW], BF, tag="Qs1")
                    nrow = min(128, PY - (r0 + 1))
                    nc.sync.dma_start(
                        Qs1[0:nrow, 1 : PX + 1],
                        h_ext[z, r0 + 1 : r0 + 1 + nrow, :],
                    )
                    if nrow < 128:
                        # fill the tail rows so no read is uninitialized
                        # (their outputs fall outside the stored row range)
                        nc.sync.dma_start(
                            Qs1[nrow:128, 1 : PX + 1],
                            h_ext[z, PY - (128 - nrow) : PY, :],
                        )
                    U = inp.tile([128, TW], BF, tag="U")
                    nc.sync.dma_start(U[:, 0:PX], u_ext[z, r0 : r0 + 128, :])
                    V_ = inp.tile([128, TW], BF, tag="V")
                    nc.sync.dma_start(V_[:, 0:PX], v_ext[z, r0 : r0 + 128, :])

                    dfex = _emit_direction_x(nc, sc, scf, wk, Qe, Qo, U)
                    dfny = _emit_direction_y(
                        nc, sc, scf, wk, psc, bands, Qe, Qo, Qs1, V_
                    )

                    oc2 = outp.tile([128, TW], F32, tag="oc2")
                    # out = dfex' + dfny' (both negated+scaled); dfny is
                    # phase 1 (valid cols [2,1025)), dfex phase 0.
                    nc.vector.tensor_tensor(
                        oc2[:, 3 : PX - 3],
                        dfny[:, 4 : PX - 2],
                        dfex[:, 3 : PX - 3],
                        ALU.add,
                    )
                    scf.free(dfny)
                    # tile row p -> global y = r0 + p - 1; rows p in [3..124]
                    gy0 = r0 + 2
                    nc.sync.dma_start(
                        o_ext[z, gy0 : gy0 + 122, 2 : NX - 2],
                        oc2[3:125, 3 : PX - 3],
                    )
    import sys
    print(
        f"build_nc: scratch_tags={sc.n}+{scf.n}f psum_tags={psc.n}",
        file=sys.stderr,
    )
    return nc


_nc_cache = {}


def _get_nc(zpc=ZPC, n_chunks=9, mode="full", repeat=1):
    key = (zpc, n_chunks, mode, repeat)
    if key not in _nc_cache:
        _nc_cache[key] = build_nc(zpc, n_chunks, mode, repeat)
    return _nc_cache[key]


def _prep_inputs(h, u, v):
    bf = mybir.dt.np(BF)
    h = np.asarray(h, dtype=np.float32)
    u = np.asarray(u, dtype=np.float32)
    v = np.asarray(v, dtype=np.float32)
    hp = np.pad(h, ((0, 0), (1, 1), (1, 1)), mode="edge").astype(bf)
    up = (
        np.pad(u, ((0, 0), (1, 1), (1, 1)), mode="edge") * np.float32(1.0 / DX)
    ).astype(bf)
    vp = (
        np.pad(v, ((0, 0), (1, 1), (1, 1)), mode="edge") * np.float32(1.0 / DY)
    ).astype(bf)
    return hp, up, vp


def make_in_maps_from(h, u, v):
    hp, up, vp = _prep_inputs(h, u, v)
    levels = list(range(1, NZ - 1)) + [NZ - 2, NZ - 2]
    bands = make_bands_host()
    in_maps = []
    for c in range(NCORES):
        lv = levels[c * ZPC : (c + 1) * ZPC]
        in_maps.append(
            {
                "h": np.ascontiguousarray(hp[lv]),
                "u": np.ascontiguousarray(up[lv]),
                "v": np.ascontiguousarray(vp[lv]),
                "bands": bands,
            }
        )
    return in_maps


def kernel(h, u, v):
    from concourse.bass_utils import run_bass_kernel_spmd

    nc = _get_nc()
    core_ids = list(range(NCORES))
    in_maps = make_in_maps_from(h, u, v)
    res = run_bass_kernel_spmd(nc, in_maps, core_ids)
    levels = list(range(1, NZ - 1)) + [NZ - 2, NZ - 2]
    out = np.zeros((NZ, NY, NX), dtype=np.float32)
    for c in core_ids:
        lv = levels[c * ZPC : (c + 1) * ZPC]
        o = np.asarray(res.results[c]["o"], dtype=np.float32)
        for j, z in enumerate(lv):
            out[z, 2 : NY - 2, 2 : NX - 2] = o[j][2 : NY - 2, 2 : NX - 2]
    return out


def profile_once(inputs):
    """Run with trace=True to extract device exec time (ns), if available."""
    from concourse.bass_utils import run_bass_kernel_spmd

    nc = _get_nc()
    core_ids = list(range(NCORES))
    in_maps = make_in_maps_from(inputs["h"], inputs["u"], inputs["v"])
    res = run_bass_kernel_spmd(nc, in_maps, core_ids, trace=True)
    return res.exec_time_ns


# revision 14
# speedup vs baseline: 4.9875x; 4.9875x over previous
"""WENO5 2D advection (Advection3D) Trainium2 kernel — 16-bit pipeline.

Full inputs h, u, v: [32, 1024, 1024] f32.  Output: same shape;
out[1:-1, 2:-2, 2:-2] = -div(WENO5 fluxes), 0 on the frame.

Sharding: z-levels across 8 cores (pure data parallel, no halo in z).
Per-core SPMD program processes ZPC=4 z-levels; each z-level swept in
y-chunks of 128 rows (122 valid output rows per chunk).

Perf-critical implementation notes:
  - DVE op mix: InstTensorScalarPtr with is_scalar_tensor_tensor has NO
    fast uops (always 1x), while tensor_tensor is 2x for 16-bit and
    tensor_scalar is 4x.  So every a*s+b op is folded into (scaled-copy via
    tensor_scalar at 4x) + (plain tensor_tensor add at 2x), constants are
    pre-folded into band-matrix coefficients and into ACT scale/bias
    (notably eps lives in the Square bias: B = (c+eps)^2), and qR uses a
    reversed subtract so no negated copies are needed.
  - 2x_1p requires every operand 4-byte aligned; each SBUF array gets a
    storage "phase" (0/1 column shift) chosen to keep hot ops aligned.
    Arrays needed at both parities (Dx, x-dir PPs) get an ACT-copy
    realignment (or DMA when dma_realign=True).
  - Narrow-range arrays (q, vel, D-layer, dl, c, t-values) are fp16
    (4x finer mantissa than bf16, same speed); wide-range arrays
    (B ~ eps^2..2.5e3^2, PP, den, rd) are bf16.
  - y-direction stencils/shifts via TensorE banded matmuls (16-bit,
    1 cyc/row); PSUM results return through ScalarE square/copy.
  - Reciprocal = exp(-ln(x)) on ScalarE + ONE bf16 Newton step: the ACT
    ln/exp table pair has ~1% relative error (hardware-verified), Newton
    squares it to the bf16 rounding floor.
  - flux tail (fa/fb/fe, divergence) in f32: flux differences are ~10x
    smaller than fluxes, 16-bit flux storage would dominate the error.
  - fn's partition shift for dfny runs on PE in f32 (separate f32 band).

Math (validated against reference):
  D_j = q_{j+1}-q_j ; A_j = D_j - D_{j-1}
  b0_j ~ c1312*A^2 + .25*(A+2D_j)^2 ; b1 ~ c1312*A^2+.25*(D_j+D_{j-1})^2
  b2 ~ c1312*A^2 + .25*(A-2D_{j-1})^2 ; B_k = (b_k+eps)^2
  PP12_j=B1_j*B2_{j+1}; PP01_j=B0_{j-1}*B1_j; PP02_j=B0_{j-1}*B2_{j+1}
  denL*10 = PP12+6*PP02+3*PP01 ; denR*10 = PP01+6*PP02+3*PP12 (R at i+1)
  corrL*(12/1.2) = PP12*dl0L' + (PP02*dl1L'' + PP01*dl2L'')  with the
    5/6 and 2.4 weights folded into dl' scales; same for R
  qL = q_i + corrL/denL' ; qR = q_{i+1} - corrR/denR'
  flux = relu(vel)*qL - relu(-vel)*qR
"""
import math

import numpy as np

import concourse.bass as bass
import concourse.mybir as mybir
import concourse.tile as tile

F32 = mybir.dt.float32
BF = mybir.dt.bfloat16
FP16 = mybir.dt.float16
ALU = mybir.AluOpType
AF = mybir.ActivationFunctionType

NZ, NY, NX = 32, 1024, 1024
NCORES = 8
ZPC = 4                      # z-levels per core (SPMD-uniform)
PY, PX = NY + 2, NX + 2      # edge-padded
TW = 1028                    # tile width
DX = 1000.0
DY = 1000.0
WENO_EPS = 1e-6
C1312S = math.sqrt(13.0 / 12.0)
CHUNK = 122                  # valid output rows per 128-row chunk
F56 = 5.0 / 6.0              # overall correction weight

DMA_REALIGN = False          # True: phase-realign copies on DMA engines


class LegalTileContext(tile.TileContext):
    """Tile + wait legalization: this walrus packs at most ONE semaphore wait
    per instruction; hoist extras onto standalone EventSemaphore instructions
    (what raw-bass wait_ge emits)."""

    def _commit_instruction(self, inst, lazy_reg_writes=True):
        si = inst.sync_info
        if si is not None and len(si.on_wait) > 1:
            waits = list(si.on_wait)
            for w in waits[:-1]:
                ev = mybir.InstEventSemaphore(
                    name=f"W-{self.nc.next_id()}", ins=[], outs=[]
                )
                ev.engine = inst.engine
                ev.sync_info = mybir.SyncInfo(on_wait=[w], on_update=[])
                if inst.debug is not None:
                    ev.debug = inst.debug
                super()._commit_instruction(ev, lazy_reg_writes=False)
            inst.sync_info = mybir.SyncInfo(
                on_wait=[waits[-1]], on_update=list(si.on_update)
            )
        return super()._commit_instruction(inst, lazy_reg_writes)

    def _drain_and_barrier(self, tick_clock, wait_clock):
        from concourse.vector_clock import ScopedClock

        nop0 = self.nc.sync.nop()
        wait_clock.add_sem_waits(
            nop0.ins, ScopedClock({None: tick_clock.global_clock})
        )
        si = nop0.ins.sync_info
        if si is not None and len(si.on_wait) > 1:
            waits = list(si.on_wait)
            nop0.ins.sync_info = mybir.SyncInfo(
                on_wait=[waits[0]], on_update=list(si.on_update)
            )
            for w in waits[1:]:
                nopk = self.nc.sync.nop()
                nopk.ins.sync_info = mybir.SyncInfo(on_wait=[w], on_update=[])
        self.nc.sync.drain()

        self.nc.all_engine_barrier()
        assert self.sems is not None
        popped = self.nc._tile_sem_poison_stack.pop()
        assert popped is self._sem_poison
        self.nc.clear_and_free_semaphores(list(self.sems.allocated().values()))
        self.nc.all_engine_barrier()


class Scratch:
    """Free-list scratch allocator.  Tags are reused only after an explicit
    free(), which callers place after the tile's last consumer is emitted —
    so slot-wait edges always point backward in emission order and can
    never form a scheduling cycle."""

    def __init__(self, pool, shape, dtype, prefix="s"):
        self.pool = pool
        self.shape = shape
        self.dtype = dtype
        self.prefix = prefix
        self.free_tags = []
        self.n = 0
        self.tag_of = {}

    def __call__(self):
        tag = self.free_tags.pop() if self.free_tags else f"{self.prefix}{self._new()}"
        t = self.pool.tile(self.shape, self.dtype, tag=tag)
        self.tag_of[id(t)] = tag
        return t

    def _new(self):
        self.n += 1
        return self.n - 1

    def free(self, *tiles):
        for t in tiles:
            self.free_tags.append(self.tag_of.pop(id(t)))


# Band matrices (lhsT layout: S[k, p] = coeff of q_k in out_p).  The
# numerator dl bands carry the folded 5/6 (overall) and 2.4 (b1/b2 vs b0)
# weights; edge rows are garbage, discarded by the final DMA row range.
BAND_SPECS = [
    ("shp1", {1: 1.0}, 1.0),                      # 0: out_p = q_{p+1}
    ("ay", {-1: 1.0, 0: -2.0, 1: 1.0}, 1.0),      # 1: A_p
    ("t0", {-1: 1.0, 0: -4.0, 1: 3.0}, 1.0),      # 2: t0_p
    ("t1", {-1: 3.0, 0: -4.0, 1: 1.0}, 1.0),      # 3: t1_p
    ("s", {-1: -1.0, 1: 1.0}, 1.0),               # 4: s_p
    ("dl0L", {-2: 0.4, -1: -1.4, 0: 1.0}, F56),   # 5
    ("dl1L", {-1: -0.5, 0: -0.5, 1: 1.0}, 2.4 * F56),  # 6
    ("dl2L", {0: -1.0, 1: 1.25, 2: -0.25}, 2.4 * F56),  # 7
    ("dl0R", {1: -1.0, 2: 1.4, 3: -0.4}, F56),    # 8
    ("dl1R", {0: -1.0, 1: 0.5, 2: 0.5}, 2.4 * F56),     # 9
    ("dl2R", {-1: 0.25, 0: -1.25, 1: 1.0}, 2.4 * F56),  # 10
    ("shm1", {-1: 1.0}, 1.0),                     # 11: out_p = q_{p-1}
]
NBANDS = len(BAND_SPECS)


def make_bands_host():
    """SBUF-layout band matrices: [128 k, NBANDS*128 cols] fp16, plus the
    f32 band-11 copy (for the f32 fn partition shift) appended as 128
    extra f32-viewed... kept separate: see make_band11_f32."""
    w = np.zeros((128, NBANDS * 128), dtype=np.float32)
    for b, (_, taps, scale) in enumerate(BAND_SPECS):
        for off, coef in taps.items():
            for p in range(128):
                k = p + off
                if 0 <= k < 128:
                    w[k, b * 128 + p] = coef * scale
    return w.astype(mybir.dt.np(FP16))


def make_band11_f32():
    w = np.zeros((128, 128), dtype=np.float32)
    for p in range(128):
        if p - 1 >= 0:
            w[p - 1, p] = 1.0
    return w


YW = 1024  # y-chain logical column count


def _newton(nc, sc, den_ap, rd0_ap, out_tile_ap, wsc):
    """One bf16 Newton step: out = rd0*(2 - den*rd0).  All APs same window,
    4B-aligned.  wsc: scratch for the two temps."""
    tn = wsc()
    nc.vector.tensor_tensor(tn, den_ap, rd0_ap, ALU.mult)
    wn = wsc()
    nc.vector.tensor_scalar(wn, tn, -1.0, 2.0, ALU.mult, ALU.add)
    nc.vector.tensor_tensor(out_tile_ap, wn, rd0_ap, ALU.mult)
    wsc.free(tn, wn)


def _emit_direction_x(nc, sc, scb, scf, wk, Qe, Qo, U):
    """X-direction WENO flux + divergence part (free-dim shifts, 16-bit).

    Phase discipline: logical column j lives at tile column j+phase; all
    2x-path operands start at even columns.  sc = fp16 scratch, scb = bf16
    scratch, scf = f32 scratch.  Returns dfex (f32, logical [3,1023) at
    cols [3,1023))."""
    tt = nc.vector.tensor_tensor
    ts = nc.vector.tensor_scalar
    act = nc.scalar.activation

    W = PX  # 1026 logical columns

    def realign(src_ap, dst_ap):
        if DMA_REALIGN:
            nc.sync.dma_start(dst_ap, src_ap)
        else:
            act(dst_ap, src_ap, AF.Copy)

    # Dx[j] = Q[j+1] - Q[j], j in [0, 1025); phase 0 (fp16).
    Dx = sc()
    tt(Dx[:, 0 : W - 1], Qo[:, 2 : W + 1], Qe[:, 0 : W - 1], ALU.subtract)
    # Dxo: phase-1 realigned copy.
    Dxo = sc()
    realign(Dx[:, 0 : W - 1], Dxo[:, 1:W])

    # b-chain precursors, phase 1, j in [1, 1025) (cols [2, 1026)):
    #   A = Dx[j]-Dx[j-1]; t0 = 2Dx[j]+A; t1 = -2Dx[j-1]+A; s = Dx[j]+Dx[j-1]
    # t0 = 3Dx[j]-Dx[j-1] and t1 = Dx[j]-3Dx[j-1], via scaled copies at 4x.
    D3e = sc()
    ts(D3e[:, 0 : W - 2], Dx[:, 0 : W - 2], 3.0, 0.0, ALU.mult, ALU.add)
    D3o = sc()
    ts(D3o[:, 2:W], Dxo[:, 2:W], 3.0, 0.0, ALU.mult, ALU.add)
    Ax = sc()
    tt(Ax[:, 2:W], Dxo[:, 2:W], Dx[:, 0 : W - 2], ALU.subtract)
    t0 = sc()
    tt(t0[:, 2:W], D3o[:, 2:W], Dx[:, 0 : W - 2], ALU.subtract)
    t1 = sc()
    tt(t1[:, 2:W], Dxo[:, 2:W], D3e[:, 0 : W - 2], ALU.subtract)
    s = sc()
    tt(s[:, 2:W], Dxo[:, 2:W], Dx[:, 0 : W - 2], ALU.add)
    sc.free(D3e, D3o)

    # squares (ACT), phase 1, fp16
    asq = sc()
    act(asq[:, 2:W], Ax[:, 2:W], AF.Square, scale=C1312S)
    sc.free(Ax)
    q0 = sc()
    act(q0[:, 2:W], t0[:, 2:W], AF.Square, scale=0.5)
    q1 = sc()
    act(q1[:, 2:W], s[:, 2:W], AF.Square, scale=0.5)
    q2 = sc()
    act(q2[:, 2:W], t1[:, 2:W], AF.Square, scale=0.5)
    sc.free(t0, t1, s)

    # c_k = asq + q_k (fp16, 2x); B_k = (c_k + eps)^2 via ACT bias -> bf16.
    c0 = sc(); tt(c0[:, 2:W], asq[:, 2:W], q0[:, 2:W], ALU.add)
    c1 = sc(); tt(c1[:, 2:W], asq[:, 2:W], q1[:, 2:W], ALU.add)
    c2 = sc(); tt(c2[:, 2:W], asq[:, 2:W], q2[:, 2:W], ALU.add)
    sc.free(asq, q0, q1, q2)
    B0 = scb()
    act(B0[:, 2:W], c0[:, 2:W], AF.Square, bias=WENO_EPS)
    B1 = scb()
    act(B1[:, 1 : W - 1], c1[:, 2:W], AF.Square, bias=WENO_EPS)
    B2 = scb()
    act(B2[:, 2:W], c2[:, 2:W], AF.Square, bias=WENO_EPS)
    sc.free(c0, c1, c2)

    # PP's at j in [2, 1024), phase 0 (bf16):
    PP12 = scb()
    tt(PP12[:, 2 : W - 2], B1[:, 2 : W - 2], B2[:, 4 : W], ALU.mult)
    PP01 = scb()
    tt(PP01[:, 2 : W - 2], B0[:, 2 : W - 2], B1[:, 2 : W - 2], ALU.mult)
    PP02 = scb()
    tt(PP02[:, 2 : W - 2], B0[:, 2 : W - 2], B2[:, 4 : W], ALU.mult)
    scb.free(B0, B1, B2)
    # phase-1 realigned copies for the R-side reads at j+1
    PP12o = scb()
    realign(PP12[:, 2 : W - 2], PP12o[:, 3 : W - 1])
    PP01o = scb()
    realign(PP01[:, 2 : W - 2], PP01o[:, 3 : W - 1])
    PP02o = scb()
    realign(PP02[:, 2 : W - 2], PP02o[:, 3 : W - 1])

    # denominators (*0.1), phase 0, j in [2, 1024), bf16:
    #   denL = 3*PP01 + (6*PP02 + PP12); denR = 3*PP12 + (6*PP02 + PP01)
    e = slice(2, W - 2)
    P6 = scb(); ts(P6[:, e], PP02[:, e], 6.0, 0.0, ALU.mult, ALU.add)
    P3a = scb(); ts(P3a[:, e], PP01[:, e], 3.0, 0.0, ALU.mult, ALU.add)
    P3b = scb(); ts(P3b[:, e], PP12[:, e], 3.0, 0.0, ALU.mult, ALU.add)
    d1 = scb(); tt(d1[:, e], P6[:, e], PP12[:, e], ALU.add)
    denL = scb(); tt(denL[:, e], P3a[:, e], d1[:, e], ALU.add)
    d2 = scb(); tt(d2[:, e], P6[:, e], PP01[:, e], ALU.add)
    denR = scb(); tt(denR[:, e], P3b[:, e], d2[:, e], ALU.add)
    scb.free(P6, P3a, P3b, d1, d2)

    # dl terms at faces i in [2, 1023), phase 0, fp16, with 5/6 and 2.4
    # weights folded into scaled-D copies:
    #   dl0L' = (5/6)(-0.4 D[i-2] + D[i-1]) = Dn13e[i-2] + D56o[i-1]
    #   dl1L'' = 2(0.5 D[i-1] + D[i])      = D1o[i-1] + D2e[i]   (x2.4*5/6)
    #   dl2L'' = 2(-0.25 D[i+1] + D[i])    = Dn5o[i+1] + D2e[i]
    #   dl0R' = Dn13e[i+2] + D56o[i+1]
    #   dl1R'' = D1o[i+1] + D2e[i] ; dl2R'' = Dn5o[i-1] + D2e[i]
    lo, hi = 2, W - 3
    F = slice(lo, hi)
    Dn13e = sc()
    ts(Dn13e[:, 0 : W - 1], Dx[:, 0 : W - 1], -F56 * 0.4, 0.0, ALU.mult, ALU.add)
    D56o = sc()
    ts(D56o[:, 1:W], Dxo[:, 1:W], F56, 0.0, ALU.mult, ALU.add)
    D2e = sc()
    ts(D2e[:, 0 : W - 1], Dx[:, 0 : W - 1], 2.0, 0.0, ALU.mult, ALU.add)
    Dn5o = sc()
    ts(Dn5o[:, 1:W], Dxo[:, 1:W], -0.5, 0.0, ALU.mult, ALU.add)
    sc.free(Dx, Dxo)
    # D56o/Dn5o written at cols [1, 1026): odd start is fine for
    # tensor_scalar only if aligned... keep writes even: rewrite below.
    dl0L = sc(); tt(dl0L[:, F], Dn13e[:, lo - 2 : hi - 2], D56o[:, lo : hi], ALU.add)
    dl1L = sc(); tt(dl1L[:, F], D1o_placeholder, D2e[:, lo:hi], ALU.add)  # noqa
    raise RuntimeError("template marker")


def build_nc(*a, **k):
    raise RuntimeError("template marker")


# revision 18
# speedup vs baseline: 10.3862x; 2.0825x over previous
"""WENO5 2D advection (Advection3D) Trainium2 kernel — 16-bit pipeline.

Full inputs h, u, v: [32, 1024, 1024] f32.  Output: same shape;
out[1:-1, 2:-2, 2:-2] = -div(WENO5 fluxes), 0 on the frame.

Sharding: z-levels across 8 cores (pure data parallel, no halo in z).
Per-core SPMD program processes ZPC=4 z-levels; each z-level swept in
y-chunks of 128 rows (122 valid output rows per chunk).

Perf-critical implementation notes:
  - DVE op mix: InstTensorScalarPtr with is_scalar_tensor_tensor has NO
    fast uops (always 1x), while tensor_tensor is 2x for 16-bit and
    tensor_scalar is up to 4x.  So every a*s+b op is decomposed into
    (scaled copy via tensor_scalar at 4x, shared across consumers) +
    (plain tensor_tensor add at 2x); constants are pre-folded into band
    coefficients and ACT scale/bias (eps lives in the Square bias:
    B = (c+eps)^2), and qR uses a reversed subtract to avoid negation.
  - 2x_1p needs every operand 4-byte aligned: each SBUF array has a
    storage "phase" (0/1 column shift) chosen to keep ops aligned.
    Arrays needed at both parities (Dx, x-dir PPs) get a DMA-engine
    realign copy; rdR gets an ACT realign.
  - Narrow-range arrays (q, vel, D-layer, dl, c, t) are fp16 (4x finer
    mantissa, same speed); wide-range (B, PP, den, rd, g) are bf16.
  - y-direction stencils/shifts via TensorE banded matmuls (fp16,
    1 cyc/row); PSUM results return through ScalarE square/copy.
  - Reciprocal = exp(-ln(x)) on ScalarE + ONE bf16 Newton step: the ACT
    ln/exp pair has ~1% relative error (hardware-verified); Newton
    squares it down to the bf16 rounding floor.
  - Flux tail (fa/fb/fe, divergence) in f32: flux differences are ~10x
    smaller than fluxes; 16-bit flux storage would dominate output error.
    fn's partition shift for dfny runs on PE in f32 (separate f32 band).

Math (validated against reference):
  D_j = q_{j+1}-q_j ; A_j = D_j - D_{j-1}
  b0 ~ c1312*A^2 + .25*(A+2D_j)^2 ; b1 ~ c1312*A^2 + .25*(D_j+D_{j-1})^2
  b2 ~ c1312*A^2 + .25*(A-2D_{j-1})^2 ; B_k = (b_k+eps)^2
  PP12_j=B1_j*B2_{j+1}; PP01_j=B0_{j-1}*B1_j; PP02_j=B0_{j-1}*B2_{j+1}
  denL = 3*PP01 + 6*PP02 + PP12 ; denR = 3*PP12 + 6*PP02 + PP01 (R at i+1)
  corrL = PP12*dl0L' + PP02*dl1L' + PP01*dl2L'   (5/6 and 2.4 folded into
  corrR = PP01'*dl0R' + PP02'*dl1R' + PP12'*dl2R'    the dl scales)
  qL = q_i + corrL/denL ; qR = q_{i+1} - corrR/denR
  flux = relu(vel)*qL - relu(-vel)*qR
"""
import math

import numpy as np

import concourse.bass as bass
import concourse.mybir as mybir
import concourse.tile as tile

F32 = mybir.dt.float32
BF = mybir.dt.bfloat16
FP16 = mybir.dt.float16
ALU = mybir.AluOpType
AF = mybir.ActivationFunctionType

NZ, NY, NX = 32, 1024, 1024
NCORES = 8
ZPC = 4                      # z-levels per core (SPMD-uniform)
PY, PX = NY + 2, NX + 2      # edge-padded
TW = 1028                    # tile width
DX = 1000.0
DY = 1000.0
WENO_EPS = 1e-6
C1312S = math.sqrt(13.0 / 12.0)
CHUNK = 122                  # valid output rows per 128-row chunk
F56 = 5.0 / 6.0              # overall correction weight

DMA_REALIGN = True           # phase-realign copies on DMA engines


class LegalTileContext(tile.TileContext):
    """Tile + wait legalization: this walrus packs at most ONE semaphore wait
    per instruction; hoist extras onto standalone EventSemaphore instructions
    (what raw-bass wait_ge emits)."""

    def _commit_instruction(self, inst, lazy_reg_writes=True):
        si = inst.sync_info
        if si is not None and len(si.on_wait) > 1:
            waits = list(si.on_wait)
            for w in waits[:-1]:
                ev = mybir.InstEventSemaphore(
                    name=f"W-{self.nc.next_id()}", ins=[], outs=[]
                )
                ev.engine = inst.engine
                ev.sync_info = mybir.SyncInfo(on_wait=[w], on_update=[])
                if inst.debug is not None:
                    ev.debug = inst.debug
                super()._commit_instruction(ev, lazy_reg_writes=False)
            inst.sync_info = mybir.SyncInfo(
                on_wait=[waits[-1]], on_update=list(si.on_update)
            )
        return super()._commit_instruction(inst, lazy_reg_writes)

    def _drain_and_barrier(self, tick_clock, wait_clock):
        from concourse.vector_clock import ScopedClock

        nop0 = self.nc.sync.nop()
        wait_clock.add_sem_waits(
            nop0.ins, ScopedClock({None: tick_clock.global_clock})
        )
        si = nop0.ins.sync_info
        if si is not None and len(si.on_wait) > 1:
            waits = list(si.on_wait)
            nop0.ins.sync_info = mybir.SyncInfo(
                on_wait=[waits[0]], on_update=list(si.on_update)
            )
            for w in waits[1:]:
                nopk = self.nc.sync.nop()
                nopk.ins.sync_info = mybir.SyncInfo(on_wait=[w], on_update=[])
        self.nc.sync.drain()

        self.nc.all_engine_barrier()
        assert self.sems is not None
        popped = self.nc._tile_sem_poison_stack.pop()
        assert popped is self._sem_poison
        self.nc.clear_and_free_semaphores(list(self.sems.allocated().values()))
        self.nc.all_engine_barrier()


class Scratch:
    """Free-list scratch allocator.  Tags are reused only after an explicit
    free(), which callers place after the tile's last consumer is emitted —
    so slot-wait edges always point backward in emission order and can
    never form a scheduling cycle."""

    def __init__(self, pool, shape, dtype, prefix="s"):
        self.pool = pool
        self.shape = shape
        self.dtype = dtype
        self.prefix = prefix
        self.free_tags = []
        self.n = 0
        self.tag_of = {}

    def __call__(self):
        tag = self.free_tags.pop() if self.free_tags else f"{self.prefix}{self._new()}"
        t = self.pool.tile(self.shape, self.dtype, tag=tag)
        self.tag_of[id(t)] = tag
        return t

    def _new(self):
        self.n += 1
        return self.n - 1

    def free(self, *tiles):
        for t in tiles:
            self.free_tags.append(self.tag_of.pop(id(t)))


# Band matrices (lhsT layout: S[k, p] = coeff of q_k in out_p).  The
# numerator dl bands carry the folded 5/6 and 2.4 weights; edge rows are
# garbage (partial sums), discarded by the final DMA row range.
BAND_SPECS = [
    ("shp1", {1: 1.0}, 1.0),                            # 0: out_p = q_{p+1}
    ("ay", {-1: 1.0, 0: -2.0, 1: 1.0}, 1.0),            # 1: A_p
    ("t0", {-1: 1.0, 0: -4.0, 1: 3.0}, 1.0),            # 2
    ("t1", {-1: 3.0, 0: -4.0, 1: 1.0}, 1.0),            # 3
    ("s", {-1: -1.0, 1: 1.0}, 1.0),                     # 4
    ("dl0L", {-2: 0.4, -1: -1.4, 0: 1.0}, F56),         # 5
    ("dl1L", {-1: -0.5, 0: -0.5, 1: 1.0}, 2.4 * F56),   # 6
    ("dl2L", {0: -1.0, 1: 1.25, 2: -0.25}, 2.4 * F56),  # 7
    ("dl0R", {1: -1.0, 2: 1.4, 3: -0.4}, F56),          # 8
    ("dl1R", {0: -1.0, 1: 0.5, 2: 0.5}, 2.4 * F56),     # 9
    ("dl2R", {-1: 0.25, 0: -1.25, 1: 1.0}, 2.4 * F56),  # 10
    ("shm1", {-1: 1.0}, 1.0),                           # 11: out_p = q_{p-1}
]
NBANDS = len(BAND_SPECS)


def make_bands_host():
    """[128 k, NBANDS*128 cols] fp16 band matrices."""
    w = np.zeros((128, NBANDS * 128), dtype=np.float32)
    for b, (_, taps, scale) in enumerate(BAND_SPECS):
        for off, coef in taps.items():
            for p in range(128):
                k = p + off
                if 0 <= k < 128:
                    w[k, b * 128 + p] = coef * scale
    return w.astype(mybir.dt.np(FP16))


def make_band11_f32():
    """f32 copy of the shm1 band for the f32 fn partition shift."""
    w = np.zeros((128, 128), dtype=np.float32)
    for p in range(1, 128):
        w[p - 1, p] = 1.0
    return w


YW = 1024  # y-chain logical column count


def _emit_chunk(nc, sc, scb, scf, wk, psc, bands, bands11f, Qe, Qo, Qs1, U, V_):
    """Emit one chunk with x- and y-direction stages interleaved so the DVE
    stream rarely waits on ACT/PE/DMA results.

    x-direction arrays: phase per comment; y-direction arrays all phase 1.
    Returns (dfex f32 phase0, dfny f32 phase1)."""
    tt = nc.vector.tensor_tensor
    ts = nc.vector.tensor_scalar
    act = nc.scalar.activation

    W = PX                   # 1026 logical x columns
    A1 = slice(2, YW + 1)    # y phase-1 cols of logical [1, 1024)
    e = slice(2, W - 2)      # x den window
    lo, hi = 2, W - 3
    F = slice(lo, hi)        # x face window (phase 0)
    Fp1 = slice(lo + 2, hi + 2)

    def realign(src_ap, dst_ap):
        if DMA_REALIGN:
            nc.sync.dma_start(dst_ap, src_ap)
        else:
            act(dst_ap, src_ap, AF.Copy)

    def pe(src, base, b):
        pt = psc()
        w2 = 512 if base == 0 else 511
        nc.tensor.matmul(
            pt[:, 0:512],
            bands[:, b * 128 : (b + 1) * 128],
            src[:, base : base + 512],
        )
        nc.tensor.matmul(
            pt[:, 512 : 512 + w2],
            bands[:, b * 128 : (b + 1) * 128],
            src[:, base + 512 : base + 512 + w2],
        )
        return pt

    def cp1(p, pool, func=AF.Copy, scale=1.0, shift=0):
        t = pool()
        w = YW - shift
        act(t[:, 1 + shift : 1 + shift + w], p[:, 0:w], func, scale=scale)
        return t

    # --- y1: banded matmuls from Q; ACT squares/copies into fp16 SBUF ---
    p = pe(Qe, 0, 1)
    yasq = cp1(p, sc, AF.Square, C1312S)
    psc.free(p)
    p = pe(Qe, 0, 2)
    yq0 = cp1(p, sc, AF.Square, 0.5)
    psc.free(p)
    p = pe(Qe, 0, 3)
    yq2 = cp1(p, sc, AF.Square, 0.5)
    psc.free(p)
    p = pe(Qe, 0, 4)
    yq1 = cp1(p, sc, AF.Square, 0.5)
    psc.free(p)
    ydls = []
    for b in (5, 6, 7):
        p = pe(Qe, 0, b)
        ydls.append(cp1(p, sc))
        psc.free(p)
    ydl0L, ydl1L, ydl2L = ydls

    # --- x1: Dx + b-chain precursors ---
    Dx = sc()
    tt(Dx[:, 0 : W - 1], Qo[:, 2 : W + 1], Qe[:, 0 : W - 1], ALU.subtract)
    Dxo = sc()
    realign(Dx[:, 0 : W - 1], Dxo[:, 1:W])
    D3e = sc()
    ts(D3e[:, 0 : W - 2], Dx[:, 0 : W - 2], 3.0, 0.0, ALU.mult, ALU.add)

    # --- y2: y c-chain on DVE (fills the Dxo realign latency) ---
    yc0 = sc(); tt(yc0[:, A1], yasq[:, A1], yq0[:, A1], ALU.add)
    yc1 = sc(); tt(yc1[:, A1], yasq[:, A1], yq1[:, A1], ALU.add)
    yc2 = sc(); tt(yc2[:, A1], yasq[:, A1], yq2[:, A1], ALU.add)
    sc.free(yasq, yq0, yq1, yq2)
    yB0 = scb(); act(yB0[:, A1], yc0[:, A1], AF.Square, bias=WENO_EPS)
    yB1 = scb(); act(yB1[:, A1], yc1[:, A1], AF.Square, bias=WENO_EPS)
    yB2 = scb(); act(yB2[:, A1], yc2[:, A1], AF.Square, bias=WENO_EPS)
    sc.free(yc0, yc1, yc2)

    # --- x2: b-chain (Dxo ready by now) ---
    D3o = sc()
    ts(D3o[:, 2:W], Dxo[:, 2:W], 3.0, 0.0, ALU.mult, ALU.add)
    Ax = sc()
    tt(Ax[:, 2:W], Dxo[:, 2:W], Dx[:, 0 : W - 2], ALU.subtract)
    t0 = sc()
    tt(t0[:, 2:W], D3o[:, 2:W], Dx[:, 0 : W - 2], ALU.subtract)
    t1 = sc()
    tt(t1[:, 2:W], Dxo[:, 2:W], D3e[:, 0 : W - 2], ALU.subtract)
    s = sc()
    tt(s[:, 2:W], Dxo[:, 2:W], Dx[:, 0 : W - 2], ALU.add)
    sc.free(D3e, D3o)
    xasq = sc()
    act(xasq[:, 2:W], Ax[:, 2:W], AF.Square, scale=C1312S)
    sc.free(Ax)
    xq0 = sc()
    act(xq0[:, 2:W], t0[:, 2:W], AF.Square, scale=0.5)
    xq1 = sc()
    act(xq1[:, 2:W], s[:, 2:W], AF.Square, scale=0.5)
    xq2 = sc()
    act(xq2[:, 2:W], t1[:, 2:W], AF.Square, scale=0.5)
    sc.free(t0, t1, s)

    # --- y3: B shifts on PE ---
    pB0 = pe(yB0, 2, 11)
    yB0m1 = cp1(pB0, scb, shift=1)
    psc.free(pB0)
    pB2 = pe(yB2, 2, 0)
    yB2p1 = cp1(pB2, scb, shift=1)
    psc.free(pB2)

    # --- x3: x c-chain + scaled-D copies (fills ACT-square latency) ---
    Dn13e = sc()
    ts(Dn13e[:, 0 : W - 1], Dx[:, 0 : W - 1], -F56 * 0.4, 0.0, ALU.mult, ALU.add)
    D56o = sc()
    ts(D56o[:, 2:W], Dxo[:, 2:W], F56, 0.0, ALU.mult, ALU.add)
    xc0 = sc(); tt(xc0[:, 2:W], xasq[:, 2:W], xq0[:, 2:W], ALU.add)
    xc1 = sc(); tt(xc1[:, 2:W], xasq[:, 2:W], xq1[:, 2:W], ALU.add)
    xc2 = sc(); tt(xc2[:, 2:W], xasq[:, 2:W], xq2[:, 2:W], ALU.add)
    sc.free(xasq, xq0, xq1, xq2)
    xB0 = scb()
    act(xB0[:, 2:W], xc0[:, 2:W], AF.Square, bias=WENO_EPS)
    xB1 = scb()
    act(xB1[:, 1 : W - 1], xc1[:, 2:W], AF.Square, bias=WENO_EPS)
    xB2 = scb()
    act(xB2[:, 2:W], xc2[:, 2:W], AF.Square, bias=WENO_EPS)
    sc.free(xc0, xc1, xc2)

    # --- y4: y PP + den (fills x B-square latency) ---
    yPP12 = scb(); tt(yPP12[:, A1], yB1[:, A1], yB2p1[:, A1], ALU.mult)
    yPP01 = scb(); tt(yPP01[:, A1], yB0m1[:, A1], yB1[:, A1], ALU.mult)
    yPP02 = scb(); tt(yPP02[:, A1], yB0m1[:, A1], yB2p1[:, A1], ALU.mult)
    scb.free(yB0, yB1, yB2, yB0m1, yB2p1)
    yP6 = scb(); ts(yP6[:, A1], yPP02[:, A1], 6.0, 0.0, ALU.mult, ALU.add)
    yP3a = scb(); ts(yP3a[:, A1], yPP01[:, A1], 3.0, 0.0, ALU.mult, ALU.add)
    yP3b = scb(); ts(yP3b[:, A1], yPP12[:, A1], 3.0, 0.0, ALU.mult, ALU.add)
    yd1 = scb(); tt(yd1[:, A1], yP6[:, A1], yPP12[:, A1], ALU.add)
    ydenL = scb(); tt(ydenL[:, A1], yP3a[:, A1], yd1[:, A1], ALU.add)
    yd2 = scb(); tt(yd2[:, A1], yP6[:, A1], yPP01[:, A1], ALU.add)
    ydenR = scb(); tt(ydenR[:, A1], yP3b[:, A1], yd2[:, A1], ALU.add)
    scb.free(yP6, yP3a, yP3b, yd1, yd2)
    ylnL = scb(); act(ylnL[:, A1], ydenL[:, A1], AF.Ln)
    yrdL0 = scb(); act(yrdL0[:, A1], ylnL[:, A1], AF.Exp, scale=-1.0)
    ylnR = scb(); act(ylnR[:, A1], ydenR[:, A1], AF.Ln)
    yrdR0 = scb(); act(yrdR0[:, A1], ylnR[:, A1], AF.Exp, scale=-1.0)
    scb.free(ylnL, ylnR)

    # --- x4: x PP + den; y R-side PP shifts on PE meanwhile ---
    xPP12 = scb()
    tt(xPP12[:, 2 : W - 2], xB1[:, 2 : W - 2], xB2[:, 4 : W], ALU.mult)
    xPP01 = scb()
    tt(xPP01[:, 2 : W - 2], xB0[:, 2 : W - 2], xB1[:, 2 : W - 2], ALU.mult)
    xPP02 = scb()
    tt(xPP02[:, 2 : W - 2], xB0[:, 2 : W - 2], xB2[:, 4 : W], ALU.mult)
    scb.free(xB0, xB1, xB2)
    xPP12o = scb()
    realign(xPP12[:, 2 : W - 2], xPP12o[:, 3 : W - 1])
    xPP01o = scb()
    realign(xPP01[:, 2 : W - 2], xPP01o[:, 3 : W - 1])
    xPP02o = scb()
    realign(xPP02[:, 2 : W - 2], xPP02o[:, 3 : W - 1])
    pP01 = pe(yPP01, 2, 0)
    yPP01s = cp1(pP01, scb, shift=1)
    psc.free(pP01)
    pP02 = pe(yPP02, 2, 0)
    yPP02s = cp1(pP02, scb, shift=1)
    psc.free(pP02)
    pP12 = pe(yPP12, 2, 0)
    yPP12s = cp1(pP12, scb, shift=1)
    psc.free(pP12)
    xP6 = scb(); ts(xP6[:, e], xPP02[:, e], 6.0, 0.0, ALU.mult, ALU.add)
    xP3a = scb(); ts(xP3a[:, e], xPP01[:, e], 3.0, 0.0, ALU.mult, ALU.add)
    xP3b = scb(); ts(xP3b[:, e], xPP12[:, e], 3.0, 0.0, ALU.mult, ALU.add)
    xd1 = scb(); tt(xd1[:, e], xP6[:, e], xPP12[:, e], ALU.add)
    xdenL = scb(); tt(xdenL[:, e], xP3a[:, e], xd1[:, e], ALU.add)
    xd2 = scb(); tt(xd2[:, e], xP6[:, e], xPP01[:, e], ALU.add)
    xdenR = scb(); tt(xdenR[:, e], xP3b[:, e], xd2[:, e], ALU.add)
    scb.free(xP6, xP3a, xP3b, xd1, xd2)
    xlnL = scb(); act(xlnL[:, e], xdenL[:, e], AF.Ln)
    xrdL0 = scb(); act(xrdL0[:, e], xlnL[:, e], AF.Exp, scale=-1.0)
    xlnR = scb(); act(xlnR[:, e], xdenR[:, e], AF.Ln)
    xrdR0 = scb(); act(xrdR0[:, e], xlnR[:, e], AF.Exp, scale=-1.0)
    scb.free(xlnL, xlnR)

    # --- y5: y numerator L + Newton (fills x ln/exp latency) ---
    yg0L = scb(); tt(yg0L[:, A1], yPP12[:, A1], ydl0L[:, A1], ALU.mult)
    yg1L = scb(); tt(yg1L[:, A1], yPP02[:, A1], ydl1L[:, A1], ALU.mult)
    yg2L = scb(); tt(yg2L[:, A1], yPP01[:, A1], ydl2L[:, A1], ALU.mult)
    sc.free(ydl0L, ydl1L, ydl2L)
    scb.free(yPP12, yPP01, yPP02)
    yn1L = scb(); tt(yn1L[:, A1], yg1L[:, A1], yg2L[:, A1], ALU.add)
    ycorrL = scb(); tt(ycorrL[:, A1], yn1L[:, A1], yg0L[:, A1], ALU.add)
    scb.free(yg0L, yg1L, yg2L, yn1L)
    ytnL = scb(); tt(ytnL[:, A1], ydenL[:, A1], yrdL0[:, A1], ALU.mult)
    ywnL = scb(); ts(ywnL[:, A1], ytnL[:, A1], -1.0, 2.0, ALU.mult, ALU.add)
    yrdL = scb(); tt(yrdL[:, A1], ywnL[:, A1], yrdL0[:, A1], ALU.mult)
    scb.free(ydenL, ytnL, ywnL, yrdL0)
    ytnR = scb(); tt(ytnR[:, A1], ydenR[:, A1], yrdR0[:, A1], ALU.mult)
    ywnR = scb(); ts(ywnR[:, A1], ytnR[:, A1], -1.0, 2.0, ALU.mult, ALU.add)
    yrdR = scb(); tt(yrdR[:, A1], ywnR[:, A1], yrdR0[:, A1], ALU.mult)
    scb.free(ydenR, ytnR, ywnR, yrdR0)
    pRd = pe(yrdR, 2, 0)
    yrdRs = cp1(pRd, scb, shift=1)
    psc.free(pRd)
    scb.free(yrdR)

    # --- x5: x dl layer + Newton ---
    D2e = sc()
    ts(D2e[:, 0 : W - 1], Dx[:, 0 : W - 1], 2.0, 0.0, ALU.mult, ALU.add)
    Dn5o = sc()
    ts(Dn5o[:, 2:W], Dxo[:, 2:W], -0.5, 0.0, ALU.mult, ALU.add)
    dl0L = sc(); tt(dl0L[:, F], Dn13e[:, lo - 2 : hi - 2], D56o[:, lo:hi], ALU.add)
    dl1L = sc(); tt(dl1L[:, F], Dxo[:, lo:hi], D2e[:, lo:hi], ALU.add)
    dl2L = sc(); tt(dl2L[:, F], Dn5o[:, Fp1], D2e[:, lo:hi], ALU.add)
    dl0R = sc(); tt(dl0R[:, F], Dn13e[:, Fp1], D56o[:, Fp1], ALU.add)
    dl1R = sc(); tt(dl1R[:, F], Dxo[:, Fp1], D2e[:, lo:hi], ALU.add)
    dl2R = sc(); tt(dl2R[:, F], Dn5o[:, lo:hi], D2e[:, lo:hi], ALU.add)
    sc.free(Dx, Dxo, Dn13e, D56o, D2e, Dn5o)
    xtnL = scb(); tt(xtnL[:, e], xdenL[:, e], xrdL0[:, e], ALU.mult)
    xwnL = scb(); ts(xwnL[:, e], xtnL[:, e], -1.0, 2.0, ALU.mult, ALU.add)
    xrdL = scb(); tt(xrdL[:, e], xwnL[:, e], xrdL0[:, e], ALU.mult)
    scb.free(xdenL, xtnL, xwnL, xrdL0)
    xtnR = scb(); tt(xtnR[:, e], xdenR[:, e], xrdR0[:, e], ALU.mult)
    xwnR = scb(); ts(xwnR[:, e], xtnR[:, e], -1.0, 2.0, ALU.mult, ALU.add)
    xrdR = scb(); tt(xrdR[:, e], xwnR[:, e], xrdR0[:, e], ALU.mult)
    scb.free(xdenR, xtnR, xwnR, xrdR0)
    xrdRo = scb()
    act(xrdRo[:, 3 : W - 1], xrdR[:, e], AF.Copy)
    scb.free(xrdR)

    # --- x6: x numerators + faces + flux ---
    g0L = scb(); tt(g0L[:, F], xPP12[:, F], dl0L[:, F], ALU.mult)
    g1L = scb(); tt(g1L[:, F], xPP02[:, F], dl1L[:, F], ALU.mult)
    g2L = scb(); tt(g2L[:, F], xPP01[:, F], dl2L[:, F], ALU.mult)
    sc.free(dl0L, dl1L, dl2L)
    n1L = scb(); tt(n1L[:, F], g1L[:, F], g2L[:, F], ALU.add)
    xcorrL = scb(); tt(xcorrL[:, F], n1L[:, F], g0L[:, F], ALU.add)
    scb.free(g0L, g1L, g2L, n1L)
    g0R = scb(); tt(g0R[:, F], xPP01o[:, Fp1], dl0R[:, F], ALU.mult)
    g1R = scb(); tt(g1R[:, F], xPP02o[:, Fp1], dl1R[:, F], ALU.mult)
    g2R = scb(); tt(g2R[:, F], xPP12o[:, Fp1], dl2R[:, F], ALU.mult)
    sc.free(dl0R, dl1R, dl2R)
    scb.free(xPP12, xPP01, xPP02, xPP12o, xPP01o, xPP02o)
    n1R = scb(); tt(n1R[:, F], g1R[:, F], g2R[:, F], ALU.add)
    xcorrR = scb(); tt(xcorrR[:, F], n1R[:, F], g0R[:, F], ALU.add)
    scb.free(g0R, g1R, g2R, n1R)
    xtL = sc(); tt(xtL[:, F], xcorrL[:, F], xrdL[:, F], ALU.mult)
    xrL = sc(); tt(xrL[:, F], xtL[:, F], Qe[:, F], ALU.add)
    scb.free(xcorrL, xrdL)
    sc.free(xtL)
    xtR = sc(); tt(xtR[:, F], xcorrR[:, F], xrdRo[:, Fp1], ALU.mult)
    xrR = sc(); tt(xrR[:, F], Qo[:, Fp1], xtR[:, F], ALU.subtract)
    scb.free(xcorrR, xrdRo)
    sc.free(xtR)
    pU = sc(); act(pU[:, F], U[:, F], AF.Relu)
    nU = sc(); act(nU[:, F], U[:, F], AF.Relu, scale=-1.0)
    xfa = scf(); tt(xfa[:, F], pU[:, F], xrL[:, F], ALU.mult)
    xfb = scf(); tt(xfb[:, F], nU[:, F], xrR[:, F], ALU.mult)
    sc.free(pU, nU, xrL, xrR)
    fe = scf(); tt(fe[:, F], xfa[:, F], xfb[:, F], ALU.subtract)
    scf.free(xfa, xfb)
    dfex = wk.tile([128, TW], F32, tag="dfex")
    tt(dfex[:, 3 : W - 3], fe[:, 2 : W - 4], fe[:, 3 : W - 3], ALU.subtract)
    scf.free(fe)

    # --- y6: y numerator R + faces + flux + dfny ---
    ydls = []
    for b in (8, 9, 10):
        p = pe(Qe, 0, b)
        ydls.append(cp1(p, sc))
        psc.free(p)
    ydl0R, ydl1R, ydl2R = ydls
    yg0R = scb(); tt(yg0R[:, A1], yPP01s[:, A1], ydl0R[:, A1], ALU.mult)
    yg1R = scb(); tt(yg1R[:, A1], yPP02s[:, A1], ydl1R[:, A1], ALU.mult)
    yg2R = scb(); tt(yg2R[:, A1], yPP12s[:, A1], ydl2R[:, A1], ALU.mult)
    sc.free(ydl0R, ydl1R, ydl2R)
    scb.free(yPP01s, yPP02s, yPP12s)
    yn1R = scb(); tt(yn1R[:, A1], yg1R[:, A1], yg2R[:, A1], ALU.add)
    ycorrR = scb(); tt(ycorrR[:, A1], yn1R[:, A1], yg0R[:, A1], ALU.add)
    scb.free(yg0R, yg1R, yg2R, yn1R)
    ytL = sc(); tt(ytL[:, A1], ycorrL[:, A1], yrdL[:, A1], ALU.mult)
    yrL = sc(); tt(yrL[:, A1], ytL[:, A1], Qo[:, A1], ALU.add)
    scb.free(ycorrL, yrdL)
    sc.free(ytL)
    ytR = sc(); tt(ytR[:, A1], ycorrR[:, A1], yrdRs[:, A1], ALU.mult)
    yrR = sc(); tt(yrR[:, A1], Qs1[:, A1], ytR[:, A1], ALU.subtract)
    scb.free(ycorrR, yrdRs)
    sc.free(ytR)
    pV = sc(); act(pV[:, A1], V_[:, 1:YW], AF.Relu)
    nV = sc(); act(nV[:, A1], V_[:, 1:YW], AF.Relu, scale=-1.0)
    yfa = scf(); tt(yfa[:, A1], pV[:, A1], yrL[:, A1], ALU.mult)
    yfb = scf(); tt(yfb[:, A1], nV[:, A1], yrR[:, A1], ALU.mult)
    sc.free(pV, nV, yrL, yrR)
    fn = scf(); tt(fn[:, A1], yfa[:, A1], yfb[:, A1], ALU.subtract)
    scf.free(yfa, yfb)
    pt = psc()
    nc.tensor.matmul(pt[:, 0:512], bands11f[:], fn[:, 2:514])
    nc.tensor.matmul(pt[:, 512:1023], bands11f[:], fn[:, 514:1025])
    dfny = scf()
    tt(dfny[:, 2:YW], pt[:, 0:1022], fn[:, 2:YW], ALU.subtract)
    psc.free(pt)
    scf.free(fn)
    return dfex, dfny


def build_nc(zpc=ZPC, n_chunks=9, mode="full", repeat=1):
    nc = bass.Bass()
    # const AP for the eps bias used by the B = (c+eps)^2 ACT squares
    eps_t = nc.alloc_sbuf_tensor("const-eps", [128, 1], F32)
    nc.gpsimd.memset(eps_t.ap(), WENO_EPS)
    nc.const_aps.aps[(F32, WENO_EPS)] = eps_t.ap()
    nc.all_engine_barrier()
    h_ext = nc.declare_dram_parameter("h", [zpc, PY, PX], FP16, isOutput=False)
    u_ext = nc.declare_dram_parameter("u", [zpc, PY, PX], FP16, isOutput=False)
    v_ext = nc.declare_dram_parameter("v", [zpc, PY, PX], FP16, isOutput=False)
    b_ext = nc.declare_dram_parameter(
        "bands", [128, NBANDS * 128], FP16, isOutput=False
    )
    b11_ext = nc.declare_dram_parameter("bands11f", [128, 128], F32, isOutput=False)
    o_ext = nc.declare_dram_parameter("o", [zpc, NY, NX], F32, isOutput=True)

    with LegalTileContext(nc) as tc:
        with (
            tc.tile_pool(name="inp", bufs=2) as inp,
            tc.tile_pool(name="wk", bufs=2) as wk,
            tc.tile_pool(name="outp", bufs=2) as outp,
            tc.tile_pool(name="bnd", bufs=1) as bnd,
            tc.tile_pool(name="ps", bufs=3, space="PSUM") as psum,
        ):
            bands = bnd.tile([128, NBANDS * 128], FP16, tag="bands")
            nc.sync.dma_start(bands[:], b_ext[:])
            bands11f = bnd.tile([128, 128], F32, tag="bands11f")
            nc.sync.dma_start(bands11f[:], b11_ext[:])
            sc = Scratch(wk, [128, TW], FP16)
            scb = Scratch(wk, [128, TW], BF, prefix="b")
            scf = Scratch(wk, [128, TW], F32, prefix="f")
            psc = Scratch(psum, [128, YW], F32, prefix="p")
            for _rep in range(repeat):
              for z in range(zpc):
                for ci in range(n_chunks):
                    r0 = CHUNK * ci
                    if r0 + 128 > PY:
                        r0 = PY - 128
                    Qe = inp.tile([128, TW], FP16, tag="Qe")
                    nc.sync.dma_start(Qe[:, 0:PX], h_ext[z, r0 : r0 + 128, :])
                    Qo = inp.tile([128, TW], FP16, tag="Qo")
                    nc.sync.dma_start(Qo[:, 1 : PX + 1], h_ext[z, r0 : r0 + 128, :])
                    # q shifted one ROW down (for qR along y), phase 1
                    Qs1 = inp.tile([128, TW], FP16, tag="Qs1")
                    nrow = min(128, PY - (r0 + 1))
                    nc.sync.dma_start(
                        Qs1[0:nrow, 1 : PX + 1],
                        h_ext[z, r0 + 1 : r0 + 1 + nrow, :],
                    )
                    if nrow < 128:
                        nc.sync.dma_start(
                            Qs1[nrow:128, 1 : PX + 1],
                            h_ext[z, PY - (128 - nrow) : PY, :],
                        )
                    U = inp.tile([128, TW], FP16, tag="U")
                    nc.sync.dma_start(U[:, 0:PX], u_ext[z, r0 : r0 + 128, :])
                    V_ = inp.tile([128, TW], FP16, tag="V")
                    nc.sync.dma_start(V_[:, 0:PX], v_ext[z, r0 : r0 + 128, :])

                    dfex, dfny = _emit_chunk(
                        nc, sc, scb, scf, wk, psc, bands, bands11f,
                        Qe, Qo, Qs1, U, V_,
                    )

                    oc2 = outp.tile([128, TW], F32, tag="oc2")
                    # out = dfex + dfny; dfny phase 1 (logical j at col j+1)
                    nc.vector.tensor_tensor(
                        oc2[:, 3 : PX - 3],
                        dfny[:, 4 : PX - 2],
                        dfex[:, 3 : PX - 3],
                        ALU.add,
                    )
                    scf.free(dfny)
                    # tile row p -> global y = r0 + p - 1; rows p in [3..124]
                    gy0 = r0 + 2
                    nc.sync.dma_start(
                        o_ext[z, gy0 : gy0 + 122, 2 : NX - 2],
                        oc2[3:125, 3 : PX - 3],
                    )
    import sys
    print(
        f"build_nc: tags={sc.n}h+{scb.n}b+{scf.n}f psum={psc.n}",
        file=sys.stderr,
    )
    return nc


_nc_cache = {}


def _get_nc(zpc=ZPC, n_chunks=9, mode="full", repeat=1):
    key = (zpc, n_chunks, mode, repeat)
    if key not in _nc_cache:
        _nc_cache[key] = build_nc(zpc, n_chunks, mode, repeat)
    return _nc_cache[key]


def _prep_inputs(h, u, v):
    f16 = mybir.dt.np(FP16)
    h = np.asarray(h, dtype=np.float32)
    u = np.asarray(u, dtype=np.float32)
    v = np.asarray(v, dtype=np.float32)
    hp = np.pad(h, ((0, 0), (1, 1), (1, 1)), mode="edge").astype(f16)
    up = (
        np.pad(u, ((0, 0), (1, 1), (1, 1)), mode="edge") * np.float32(1.0 / DX)
    ).astype(f16)
    vp = (
        np.pad(v, ((0, 0), (1, 1), (1, 1)), mode="edge") * np.float32(1.0 / DY)
    ).astype(f16)
    return hp, up, vp


def make_in_maps_from(h, u, v):
    hp, up, vp = _prep_inputs(h, u, v)
    levels = list(range(1, NZ - 1)) + [NZ - 2, NZ - 2]
    bands = make_bands_host()
    b11 = make_band11_f32()
    in_maps = []
    for c in range(NCORES):
        lv = levels[c * ZPC : (c + 1) * ZPC]
        in_maps.append(
            {
                "h": np.ascontiguousarray(hp[lv]),
                "u": np.ascontiguousarray(up[lv]),
                "v": np.ascontiguousarray(vp[lv]),
                "bands": bands,
                "bands11f": b11,
            }
        )
    return in_maps


def kernel(h, u, v):
    from concourse.bass_utils import run_bass_kernel_spmd

    nc = _get_nc()
    core_ids = list(range(NCORES))
    in_maps = make_in_maps_from(h, u, v)
    res = run_bass_kernel_spmd(nc, in_maps, core_ids)
    levels = list(range(1, NZ - 1)) + [NZ - 2, NZ - 2]
    out = np.zeros((NZ, NY, NX), dtype=np.float32)
    for c in core_ids:
        lv = levels[c * ZPC : (c + 1) * ZPC]
        o = np.asarray(res.results[c]["o"], dtype=np.float32)
        for j, z in enumerate(lv):
            out[z, 2 : NY - 2, 2 : NX - 2] = o[j][2 : NY - 2, 2 : NX - 2]
    return out


def profile_once(inputs):
    from concourse.bass_utils import run_bass_kernel_spmd

    nc = _get_nc()
    core_ids = list(range(NCORES))
    in_maps = make_in_maps_from(inputs["h"], inputs["u"], inputs["v"])
    res = run_bass_kernel_spmd(nc, in_maps, core_ids, trace=True)
    return res.exec_time_ns
